# revision 41
# baseline (speedup 1.0000x reference)
"""8-core data-parallel fused attention kernel for TRN2 (Bass/Tile).

Problem: B=8, N=1024 (32x32 grid), DIM=1024, 16 heads x 64, axial RoPE on
first 32 channels of each head, softmax attention, output projection.

Sharding: pure data-parallel -- core b computes batch element b end-to-end.
No collectives.

v2 design (exploits measured PE tile concurrency: two tile_position row
bands OR col bands with independent moving-data streams run fully
concurrently at 1 col/cycle each):

- All matmuls bf16 (PSUM f32). QKV computed transposed (features on
  partitions) in per-head-pair blocks: [h_even rot32; h_even pass32;
  h_odd rot32; h_odd pass32] so scores are single K=64-contract matmuls.
- rotate_half via a 128x128 +-1 pair-swap matrix PM on the *activations*
  (1 matmul per tile) instead of extra QKV weight columns (8 matmuls):
  q2 = qkv*cosF + PM @ (qkv*sinF); legal because cos/sin are pairwise
  constant along channels. cosF/sinF have 1/0 rows on pass channels.
- scores^T per (pair, kc): two concurrent K=64 matmuls at row bands
  (0,0)/(64,0) into one [128,1024] PSUM pair-tile; one Exp over both.
- attn@V: V matmuls for the two heads col-packed at (0,0)/(0,64) (M=64
  each, one PSUM bank for both heads); softmax denominators from a
  parallel pair of ones[128,64] matmuls, also col-packed, accumulating
  into a second bank as per-partition-broadcast rows. Normalization is
  one reciprocal + one tensor_tensor for both heads. No DRAM bounce.
- Emission weaves QKV/V/proj streams into the exp-gated attention stream
  as PE filler (in-order engine queues: emission order = execution order).
- Output written bf16 (0.4% quantization, budget is 2e-2), 2-way
  ring-split per tile, so the kernel tail isn't dominated by one
  ~16 GB/s DMA ring draining f32.

- Softmax denominators: exp tiles pre-summed pairwise on the DVE
  (den = ones @ sum(aT) is exact since the ones stationary is
  chunk-invariant), halving the den matmul streams.

Measured on trn2 (8 cores, axon): HW exec 271605 ns, rel err 5.0e-3
(baseline v1: ~347 us / 328 us quoted, rel err 4.7e-3). Note: the
device p-state sometimes caps at ~2.0 GHz (matmul streams 259 ns vs
216 ns), inflating whole-run measurements ~1.2x.
"""

import os
import sys

for _p in ("/opt/trn_rl_repo",):
    if os.path.isdir(_p) and _p not in sys.path:
        sys.path.insert(0, _p)

import numpy as np
import ml_dtypes

import concourse.bass as bass
import concourse.bacc as bacc
import concourse.mybir as mybir
import concourse.tile as tile
from concourse.bass_utils import run_bass_kernel_spmd

P = 128
NTOK = 1024
DIM = 1024
HEADS = 16
HD = 64
ROT = 32
QT = 512          # free-dim tile for matmuls (one PSUM bank of f32)
NQ = NTOK // QT   # 2
NPAIR = 8
BF = mybir.dt.bfloat16
F32 = mybir.dt.float32
AL = mybir.AluOpType
AF = mybir.ActivationFunctionType

LAST_RESULT = None
_BUILT = None


# ---------------------------------------------------------------- host prep

def _axial_tables():
    """cos/sin[t, d] for t=0..1023 (t=h*32+w), d=0..31, exactly as reference."""
    rot_half = 8
    base = np.linspace(1.0, 512.0, rot_half) * np.pi          # (8,)
    th = np.linspace(-1.0, 1.0, 32)[:, None] * base[None, :]  # (32, 8)
    fh = np.repeat(th, 2, axis=-1)                            # (32, 16)
    freqs = np.zeros((32, 32, ROT))
    freqs[:, :, :16] = fh[:, None, :]                         # H-axis channels
    freqs[:, :, 16:] = fh[None, :, :]                         # W-axis channels
    f = freqs.reshape(NTOK, ROT)
    return np.cos(f).astype(np.float32), np.sin(f).astype(np.float32)


def _prep_weights(Wqkv, Wproj, bproj):
    Wq, Wk, Wv = Wqkv[0:DIM], Wqkv[DIM:2 * DIM], Wqkv[2 * DIM:3 * DIM]
    # per-pair feature blocks: [h_even 64ch; h_odd 64ch] for Q then K.
    blocks = []
    for pr in range(NPAIR):
        for W in (Wq, Wk):
            blocks.append(W[2 * pr * HD:(2 * pr + 2) * HD])   # (128, 1024)
    wqk = np.concatenate(blocks, axis=0)                      # (2048, 1024)

    cos_td, sin_td = _axial_tables()                          # (1024, 32)
    cosF = np.ones((P, NTOK), np.float32)
    sinF = np.zeros((P, NTOK), np.float32)
    cosF[0:32] = cos_td.T
    cosF[64:96] = cos_td.T
    sinF[0:32] = sin_td.T
    sinF[64:96] = sin_td.T

    # pair-swap/negate matrix: psT[j] = sign_j * u[j^1] on rot rows, 0 on pass
    PM = np.zeros((P, P), np.float32)
    for j in range(P):
        if j % HD < ROT:
            PM[j ^ 1, j] = -1.0 if j % 2 == 0 else 1.0

    biasT = bproj.reshape(8, P).T.copy()                      # (128, 8)
    bf = ml_dtypes.bfloat16
    return {
        "wqk": np.ascontiguousarray(wqk.T).astype(bf),        # (1024, 2048)
        "wv": np.ascontiguousarray(Wv.T).astype(bf),          # (1024, 1024)
        "wp": np.ascontiguousarray(Wproj.T).astype(bf),       # (1024, 1024)
        "cosf": np.ascontiguousarray(cosF).astype(bf),
        "sinf": np.ascontiguousarray(sinF).astype(bf),
        "pm": np.ascontiguousarray(PM).astype(bf),            # (128, 128)
        "biasT": np.ascontiguousarray(biasT.astype(np.float32)),
    }


# ------------------------------------------------------------- bass builder

def _build():
    nc = bacc.Bacc()
    xT_e = nc.declare_dram_parameter("xT", [DIM, NTOK], BF, isOutput=False)
    wqk_e = nc.declare_dram_parameter("wqk", [DIM, 2 * DIM], BF, isOutput=False)
    wv_e = nc.declare_dram_parameter("wv", [DIM, DIM], BF, isOutput=False)
    wp_e = nc.declare_dram_parameter("wp", [DIM, DIM], BF, isOutput=False)
    cos_e = nc.declare_dram_parameter("cosf", [P, NTOK], BF, isOutput=False)
    sin_e = nc.declare_dram_parameter("sinf", [P, NTOK], BF, isOutput=False)
    pm_e = nc.declare_dram_parameter("pm", [P, P], BF, isOutput=False)
    b_e = nc.declare_dram_parameter("biasT", [P, 8], F32, isOutput=False)
    out_e = nc.declare_dram_parameter("out", [DIM, NTOK], BF, isOutput=True)

    with tile.TileContext(nc) as tc:
        with (
            tc.tile_pool(name="persist", bufs=1) as persist,
            tc.tile_pool(name="work", bufs=3) as work,
            tc.tile_pool(name="work3", bufs=12) as work3,
            tc.tile_pool(name="ps_sc", bufs=2, space="PSUM") as ps_sc_pool,
            tc.tile_pool(name="ps_av", bufs=1, space="PSUM") as ps_av_pool,
            tc.tile_pool(name="ps_den", bufs=1, space="PSUM") as ps_den_pool,
            tc.tile_pool(name="ps_mm", bufs=2, space="PSUM") as ps_mm_pool,
        ):
            xT = persist.tile([P, 8, NTOK], BF)
            wqk = persist.tile([P, 8, 2 * DIM], BF)
            wv = persist.tile([P, 8, DIM], BF)
            wp = persist.tile([P, 8, DIM], BF)
            cosF = persist.tile([P, NTOK], BF)
            sinF = persist.tile([P, NTOK], BF)
            pm = persist.tile([P, P], BF)
            ones64 = persist.tile([P, HD], BF)
            biasT = persist.tile([P, 8], F32)
            # roped QK, pair-stacked: partitions = [rot_e, pass_e, rot_o,
            # pass_o], chunk = pair index
            q2 = persist.tile([P, NPAIR, NTOK], BF)
            k2 = persist.tile([P, NPAIR, NTOK], BF)
            # V natural: [k-token partitions, kc, head*64+d]
            v = persist.tile([P, 8, DIM], BF)
            # attention out, transposed: partition 64*(h%2)+d, chunk h//2
            outT = persist.tile([P, 8, NTOK], BF)

            # ---------------- PE warmup: the clock ramps 0.65->2.4 GHz only
            # after ~3us of continuous busy. Spin dependency-free matmuls on
            # memset scratch during the DMA lead-in so real work starts at
            # full clock. Result is never read.
            wup = persist.tile([P, QT], BF)
            nc.vector.memset(wup[:], 0.5)
            pw = ps_mm_pool.tile([P, QT], F32, tag="ps_mm")
            for wi in range(24):
                nc.tensor.matmul(pw[:], wup[:, 0:P], wup[:],
                                 start=(wi == 0), stop=(wi == 23))

            # ---------------- input DMAs. First QKV unit (pair 0, Q) needs
            # wqk block 0 + xT t2=0; interleave so they land first.
            for cc in range(8):
                nc.sync.dma_start(out=xT[:, cc, 0:QT],
                                  in_=xT_e[cc * P:(cc + 1) * P, 0:QT])
                nc.sync.dma_start(
                    out=wqk[:, cc, 0:2 * P],
                    in_=wqk_e[cc * P:(cc + 1) * P, 0:2 * P])
            nc.sync.dma_start(out=cosF[:], in_=cos_e[:, :])
            nc.sync.dma_start(out=sinF[:], in_=sin_e[:, :])
            nc.sync.dma_start(out=pm[:], in_=pm_e[:, :])
            for cc in range(8):
                nc.sync.dma_start(out=xT[:, cc, QT:NTOK],
                                  in_=xT_e[cc * P:(cc + 1) * P, QT:NTOK])
            for cc in range(8):
                nc.sync.dma_start(out=wv[:, cc, 0:QT],
                                  in_=wv_e[cc * P:(cc + 1) * P, 0:QT])
            for cc in range(8):
                nc.sync.dma_start(
                    out=wqk[:, cc, 2 * P:4 * P],
                    in_=wqk_e[cc * P:(cc + 1) * P, 2 * P:4 * P])
            for cc in range(8):
                nc.sync.dma_start(out=wv[:, cc, QT:DIM],
                                  in_=wv_e[cc * P:(cc + 1) * P, QT:DIM])
            for cc in range(8):
                nc.sync.dma_start(
                    out=wqk[:, cc, 4 * P:8 * P],
                    in_=wqk_e[cc * P:(cc + 1) * P, 4 * P:8 * P])
            for cc in range(8):
                nc.sync.dma_start(
                    out=wqk[:, cc, 8 * P:16 * P],
                    in_=wqk_e[cc * P:(cc + 1) * P, 8 * P:16 * P])
            nc.vector.memset(ones64[:], 1.0)
            nc.sync.dma_start(out=biasT[:], in_=b_e[:, :])
            for cc in range(8):
                nc.sync.dma_start(out=wp[:, cc, :], in_=wp_e[cc * P:(cc + 1) * P, :])

            # ---------------- QKV^T + RoPE epilogue, software-pipelined:
            # block i's swap-matmul + final add are deferred until after
            # block i+1's main matmuls so the PE never waits on the DVE
            # mults feeding the swap.
            def qkv_stream(pairs, t2_outer=False):
                pends = []
                if t2_outer:
                    order = [(pr, t2) for t2 in range(NQ) for pr in pairs]
                else:
                    order = [(pr, t2) for pr in pairs for t2 in range(NQ)]
                for pr, t2 in order:
                    if True:
                        for which in range(2):
                            blk = 2 * pr + which
                            dst = q2 if which == 0 else k2
                            ts_ = slice(t2 * QT, (t2 + 1) * QT)
                            psA = ps_mm_pool.tile([P, QT], F32, tag="ps_mm")
                            for cc in range(8):
                                nc.tensor.matmul(
                                    psA[:],
                                    wqk[:, cc, blk * P:(blk + 1) * P],
                                    xT[:, cc, ts_],
                                    start=(cc == 0), stop=(cc == 7))
                            yield
                            if pends:
                                pends.pop(0)()
                            t1 = work.tile([P, QT], BF, tag="t1")
                            u = work.tile([P, QT], BF, tag="u")
                            nc.vector.tensor_tensor(
                                t1[:], psA[:], cosF[:, ts_], op=AL.mult)
                            nc.vector.tensor_tensor(
                                u[:], psA[:], sinF[:, ts_], op=AL.mult)

                            def mk(dst, pr, ts_, t1, u):
                                def emit():
                                    psT = ps_mm_pool.tile(
                                        [P, QT], F32, tag="ps_mm")
                                    nc.tensor.matmul(psT[:], pm[:], u[:],
                                                     start=True, stop=True)
                                    nc.vector.tensor_add(
                                        dst[:, pr, ts_], t1[:], psT[:])
                                return emit

                            pends.append(mk(dst, pr, ts_, t1, u))
                            yield
                for pd in pends:
                    pd()

            # ---------------- V = x @ Wv^T, natural orientation
            def v_units(g):
                for tt in range(8):
                    pt = ps_mm_pool.tile([P, QT], F32, tag="ps_mm")
                    for cc in range(8):
                        nc.tensor.matmul(
                            pt[:],
                            xT[:, cc, tt * P:(tt + 1) * P],
                            wv[:, cc, g * QT:(g + 1) * QT],
                            start=(cc == 0), stop=(cc == 7))
                    nc.vector.tensor_copy(v[:, tt, g * QT:(g + 1) * QT], pt[:])
                    yield

            # ---------------- attention for one pair (2 heads) x one q-tile
            # Col-packed attn@V: V matmuls for h0/h1 at col tiles
            # (0,0)/(0,64). The denominator ones-matmul chains use the SAME
            # positions, so they are emitted only after the V chains have
            # fully closed -- interleaving two open accumulation chains at
            # one tile position corrupts PSUM (measured).
            def attn_pair(pr, qt):
                qs = slice(qt * QT, (qt + 1) * QT)
                h0 = 2 * pr
                h1 = h0 + 1
                po = ps_av_pool.tile([P, QT], F32, tag="ps_po")
                den = ps_den_pool.tile([P, QT], F32, tag="ps_den")
                all_aTs = []

                for kc in range(8):
                    ks = slice(kc * P, (kc + 1) * P)
                    psS = ps_sc_pool.tile([P, 2 * QT], F32, tag="ps_s")
                    nc.tensor.matmul(
                        psS[:, 0:QT], k2[0:HD, pr, ks], q2[0:HD, pr, qs],
                        start=True, stop=True, tile_position=(0, 0))
                    nc.tensor.matmul(
                        psS[:, QT:2 * QT], k2[HD:P, pr, ks],
                        q2[HD:P, pr, qs],
                        start=True, stop=True, tile_position=(64, 0))
                    aT = work3.tile([P, 2 * QT], BF, tag="aT", bufs=10)
                    nc.scalar.activation(aT[:], psS[:], AF.Exp, scale=0.125)
                    all_aTs.append(aT)
                    if kc % 2 == 1:
                        yield
                # pairwise DVE pre-sums: den = ones @ sum_kc aT_kc, and the
                # ones stationary is chunk-invariant, so summing aT pairs
                # first halves the den matmul streams.
                sums = []
                for j in range(4):
                    sm = work3.tile([P, 2 * QT], BF, tag="aTs", bufs=6)
                    nc.vector.tensor_add(
                        sm[:], all_aTs[2 * j][:], all_aTs[2 * j + 1][:])
                    sums.append(sm)
                for kc, aT in enumerate(all_aTs):
                    st = (kc == 0)
                    sp = (kc == 7)
                    nc.tensor.matmul(
                        po[0:HD, :], v[:, kc, h0 * HD:(h0 + 1) * HD],
                        aT[:, 0:QT], start=st, stop=sp,
                        tile_position=(0, 0))
                    nc.tensor.matmul(
                        po[HD:P, :], v[:, kc, h1 * HD:(h1 + 1) * HD],
                        aT[:, QT:2 * QT], start=st, stop=sp,
                        tile_position=(0, 64))
                    if kc == 3:
                        yield
                yield
                for j, sm in enumerate(sums):
                    st = (j == 0)
                    sp = (j == 3)
                    nc.tensor.matmul(
                        den[0:HD, :], ones64[:], sm[:, 0:QT],
                        start=st, stop=sp, tile_position=(0, 0))
                    nc.tensor.matmul(
                        den[HD:P, :], ones64[:], sm[:, QT:2 * QT],
                        start=st, stop=sp, tile_position=(0, 64))
                yield
                rd = work.tile([P, QT], F32, tag="rd")
                nc.vector.reciprocal_approx_fast(rd[:], den[:])
                nc.vector.tensor_tensor(
                    outT[:, pr, qs], po[:], rd[:], op=AL.mult)
                yield

            # ---------------- output projection + bias
            def proj_units(qt):
                qs = slice(qt * QT, (qt + 1) * QT)
                for ot in range(8):
                    os_ = slice(ot * P, (ot + 1) * P)
                    pt = ps_mm_pool.tile([P, QT], F32, tag="ps_mm")
                    for cc in range(8):
                        nc.tensor.matmul(
                            pt[:], wp[:, cc, os_], outT[:, cc, qs],
                            start=(cc == 0), stop=(cc == 7))
                    # bf16 output (harness casts back; error budget 2e-2
                    # dwarfs the 0.4% quantization) halves the output-DMA
                    # bytes; 2-way ring split keeps the tail short.
                    ys = work.tile([P, QT], BF, tag="ys")
                    nc.vector.tensor_scalar_add(ys[:], pt[:], biasT[:, ot:ot + 1])
                    nc.sync.dma_start(out=out_e[ot * P:ot * P + HD, qs],
                                      in_=ys[0:HD])
                    nc.sync.dma_start(out=out_e[ot * P + HD:(ot + 1) * P, qs],
                                      in_=ys[HD:P])
                    yield

            def run(gen):
                for _ in gen:
                    pass

            def weave(a, b, ra=2, rb=1):
                """Generator: alternate ra units from a with rb units from b."""
                a, b = iter(a), iter(b)
                alive_a = alive_b = True
                while alive_a or alive_b:
                    for _ in range(ra):
                        if alive_a:
                            try:
                                next(a)
                            except StopIteration:
                                alive_a = False
                            else:
                                yield
                    for _ in range(rb):
                        if alive_b:
                            try:
                                next(b)
                            except StopIteration:
                                alive_b = False
                            else:
                                yield

            def chain(*gens):
                for g in gens:
                    for _ in g:
                        yield

            # schedule: qkv pairs 0-1 woven with V(g0) up front; attention
            # qt=0 wave woven with remaining qkv + V(g1) at a ratio that
            # keeps every producer strictly ahead of its consumer in
            # emission order; qt=1 wave woven with proj(0); proj(1) last.
            run(weave(qkv_stream([0, 1]), v_units(0), 8, 4))
            filler0 = chain(qkv_stream([2, 3, 4]), v_units(1),
                            qkv_stream([5, 6, 7]))
            attn0 = chain(*[attn_pair(pr, 0) for pr in range(NPAIR)])
            run(weave(attn0, filler0, 1, 1))
            attn1 = chain(*[attn_pair(pr, 1) for pr in range(NPAIR)])
            run(weave(attn1, proj_units(0), 8, 1))
            run(proj_units(1))

    nc.compile()
    return nc


def _get_nc():
    global _BUILT
    if _BUILT is None:
        _BUILT = _build()
    return _BUILT


# ------------------------------------------------- tracing support (axon)

def _ensure_trace_hooks():
    """Register the NTFF profile hook that the bare agent image's antenv
    stub lacks, and neuter the artifact upload (no bucket in-container)."""
    import types
    import concourse.bass_utils as bu

    bu.upload_artifacts = lambda tmpdir: f"local:{tmpdir}"
    try:
        from antenv.axon_hooks import get_axon_ntff_profile_hook  # noqa: F401
        return
    except ImportError:
        pass
    mod = types.ModuleType("antenv.axon_hooks")
    _state = {"hook": None}
    mod.set_axon_ntff_profile_hook = lambda h: _state.__setitem__("hook", h)
    mod.get_axon_ntff_profile_hook = lambda: _state["hook"]
    import antenv
    sys.modules["antenv.axon_hooks"] = mod
    antenv.axon_hooks = mod
    try:
        from trn_agent_boot.trn_boot import _ntff_profile_via_ctypes
        hook = _ntff_profile_via_ctypes("/opt/axon/libaxon_pjrt.so")
        if hook is not None:
            mod.set_axon_ntff_profile_hook(hook)
    except Exception as e:  # pragma: no cover
        print(f"NTFF hook install failed: {e!r}")


# ----------------------------------------------------------------- kernel()

def kernel(x, Wqkv, Wproj, bproj):
    global LAST_RESULT
    x = np.asarray(x, np.float32)
    Wqkv = np.asarray(Wqkv, np.float32)
    Wproj = np.asarray(Wproj, np.float32)
    bproj = np.asarray(bproj, np.float32)
    B = x.shape[0]

    base = _prep_weights(Wqkv, Wproj, bproj)
    bf = ml_dtypes.bfloat16
    in_maps = [
        dict(base, xT=np.ascontiguousarray(x[b].T).astype(bf)) for b in range(B)
    ]
    nc = _get_nc()
    trace = bool(os.environ.get("KBENCH_TRACE"))
    if trace:
        _ensure_trace_hooks()
    res = run_bass_kernel_spmd(
        nc, in_maps, core_ids=list(range(B)), trace=trace)
    LAST_RESULT = res
    out = np.stack([np.asarray(res.results[b]["out"], np.float32).T
                    for b in range(B)])
    return np.ascontiguousarray(out.astype(np.float32))
